# revision 8
# baseline (speedup 1.0000x reference)
"""Trainium2 Bass kernel for nn_GCL (GNN message-passing layer), 8-core SPMD.

Sharding: edges sharded by destination node (col) range; each core owns
N/8 = 6250 nodes and all edges pointing at them. Within a core, edges are
split into 2 groups by source-row half (so gather indices fit int16), each
group sorted by col. BatchNorm batch stats are all-reduced across cores.

Device pipeline per core:
  P1  per 512-edge chunk: 4 transposed bf16 dma_gathers (A[col], B[row] as
      exact hi/lo bf16 pairs) -> identity-matmul PSUM accumulation
      + edge_attr @ W1c (hi/lo bf16 matmuls) -> Silu (ScalarE, accum_out
      gives BN sum) -> Square (accum_out gives BN sqsum) -> x to DRAM.
  AR  tiny AllReduce of [128,2] BN stats; fold into per-feature scale/bias.
  P2  reload x -> fused BN-affine+Silu -> ew2 matmul -> Silu -> attention
      (per-subchunk matmul + Sigmoid) -> PE transpose -> gate -> one-hot
      matmul scatter into PSUM -> dynamic-offset add into SBUF agg.
  P3  node MLP feature-major + residual; output transposed per core.

A = h @ W1a.T + eb1 and B = h @ W1b.T are per-node precomputes (host, exact
fp32, shipped as hi/lo bf16 pairs: error ~2^-17). All 1/0.6 activation
scales are folded into BN/eps/weights exactly (BN is scale-invariant).
"""
import os
import numpy as np
import ml_dtypes

import concourse.bass as bass
import concourse.bacc as bacc
import concourse.mybir as mybir
import concourse.tile as tile
from concourse.bass_utils import run_bass_kernel_spmd
from concourse.masks import make_identity

F32 = mybir.dt.float32
F32R = mybir.dt.float32r
BF16 = mybir.dt.bfloat16
I16 = mybir.dt.int16
I32 = mybir.dt.int32
AF = mybir.ActivationFunctionType
OP = mybir.AluOpType

NC_CORES = 8
P = 128
CH = 512          # edges per chunk
SUB = 128         # edges per subchunk
BN_EPS = 1e-5
NORM = 100.0
SS = 1.0 / 0.6    # scaled-silu factor

EW2_F32R = os.environ.get("EW2_F32R", "0") == "1"
SIM_MODE = os.environ.get("BASS_KERNEL_SIM", "0") == "1"


def _hilo(x):
    hi = x.astype(ml_dtypes.bfloat16)
    lo = (x - hi.astype(np.float32)).astype(ml_dtypes.bfloat16)
    return hi, lo


def _silu(x):
    return x / (1.0 + np.exp(-x))


def _pack_idx(idx):
    """int16 [n] -> [128, n//16] wrapped in 16 partitions, replicated x8."""
    n = idx.shape[0]
    w = idx.reshape(n // 16, 16).T            # [16, n//16]
    return np.tile(w, (8, 1)).astype(np.int16)


def prep_core(c, col, row, order_all, h, edge_attr, A_full, B_full, G, NPC, HALF, NODES_PAD):
    """Build per-core host tensors. order_all: edge ids for core c,
    list of 2 arrays (per group), each sorted by col."""
    S = 2 * G
    nch_g = G // CH
    colloc = np.full(S, 0, np.int64)
    rowloc = np.zeros(S, np.int64)
    col_rel = np.full(S, -1000.0, np.float64)
    bases = np.zeros(2 * nch_g, np.int64)
    n_pad = np.zeros(2, np.int64)
    ea_s = np.zeros((S, edge_attr.shape[1]), np.float32)
    for g in (0, 1):
        ids = order_all[g]
        n = ids.shape[0]
        o = g * G
        n_pad[g] = G - n
        colloc[o:o + n] = col[ids] - c * NPC
        rowloc[o:o + n] = row[ids] - g * HALF
        ea_s[o:o + n] = edge_attr[ids]
        for k in range(nch_g):
            s = o + k * CH
            ncheck = min(n - k * CH, CH)
            if ncheck <= 0:
                base = 0
            else:
                base = min(colloc[s], NODES_PAD - SUB)
                span = colloc[s + ncheck - 1] - base
                assert span < P, f"chunk col_rel {span} >= {P}"
            bases[g * nch_g + k] = base
            col_rel[s:s + CH] = colloc[s:s + CH] - base
        col_rel[o + n:o + G] = -1000.0
    # pads gather table row 0
    a_idx = colloc.copy()
    b_idx = rowloc.copy()
    for g in (0, 1):
        o = g * G
        n = G - int(n_pad[g])
        a_idx[o + n:o + G] = 0
        b_idx[o + n:o + G] = 0

    # exact stats corrections for pad edges (mimic device op order)
    corr_sum = np.zeros(P, np.float32)
    corr_sq = np.zeros(P, np.float32)
    Ahi, Alo = _hilo(A_full[c * NPC:(c + 1) * NPC])
    Bhi, Blo = _hilo(B_full)
    for g in (0, 1):
        if n_pad[g] == 0:
            continue
        a0 = Ahi[0].astype(np.float32) + Alo[0].astype(np.float32)
        b0 = (Bhi[g * HALF].astype(np.float32) + Blo[g * HALF].astype(np.float32))
        xp = _silu((a0 + b0).astype(np.float32)).astype(np.float32)
        corr_sum += n_pad[g] * xp
        corr_sq += n_pad[g] * xp * xp

    eahi, ealo = _hilo(ea_s)
    return dict(
        a_idx=_pack_idx(a_idx.astype(np.int16)),
        b_idx=_pack_idx(b_idx.astype(np.int16)),
        col4=col_rel.reshape(S // SUB, SUB).T.astype(np.float32).copy(),  # [128, nsub]
        bases=bases[None, :].astype(np.int32),
        eahiT=np.ascontiguousarray(eahi.T),      # [16, S] bf16
        ealoT=np.ascontiguousarray(ealo.T),
        Ahi=np.ascontiguousarray(Ahi), Alo=np.ascontiguousarray(Alo),
        corr=np.stack([corr_sum, corr_sq], 1),   # [128, 2]
    )


def build_nc(G, NPC, NODES_PAD, E_total, HALF_T):
    S = 2 * G
    NCHT = S // CH
    NSUB = S // SUB
    nc = bacc.Bacc("TRN2", target_bir_lowering=False, debug=False,
                   num_devices=NC_CORES)
    t_in = {}
    for nm, shp, dt in [
        ("Ahi", [NPC, P], BF16), ("Alo", [NPC, P], BF16),
        ("Bhi", [2 * HALF_T, P], BF16), ("Blo", [2 * HALF_T, P], BF16),
        ("eahiT", [16, S], BF16), ("ealoT", [16, S], BF16),
        ("a_idx", [P, NSUB * 8], I16), ("b_idx", [P, NSUB * 8], I16),
        ("col4", [P, NSUB], F32), ("bases", [1, NCHT], I32),
        ("corr", [P, 2], F32),
        ("w1chiT", [16, P], BF16), ("w1cloT", [16, P], BF16),
        ("ew2T", [P, P], F32), ("awcol", [P, 1], F32),
        ("nw1aT", [P, P], F32), ("nw1bT", [P, P], F32), ("nw2T", [P, P], F32),
        ("pvec", [P, 6], F32),   # 0 gamma,1 beta,2 eb2,3 nb1,4 nb2,5 ab_rep
        ("hT", [P, NODES_PAD], F32),
    ]:
        t_in[nm] = nc.dram_tensor(nm, shp, dt, kind="ExternalInput")
    outT = nc.dram_tensor("outT", [P, NODES_PAD], F32, kind="ExternalOutput")
    xbuf = nc.dram_tensor("xbuf", [P, S], F32)

    ew2_dt = F32R if EW2_F32R else F32

    with tile.TileContext(nc) as tc:
        with (
            tc.tile_pool(name="cst", bufs=1) as cst,
            tc.tile_pool(name="res", bufs=1) as res,
            tc.tile_pool(name="wrk", bufs=3) as wrk,
            tc.tile_pool(name="dram", bufs=1, space="DRAM") as dr,
        ):
            # ---- constants / resident tensors ----
            ident_b = cst.tile([P, P], BF16)
            make_identity(nc, ident_b[:])
            ident_f = cst.tile([P, P], F32)
            make_identity(nc, ident_f[:])
            iota_i = cst.tile([P, SUB], I32)
            nc.gpsimd.iota(iota_i[:], pattern=[[1, SUB]], base=0,
                           channel_multiplier=0)
            iota4 = cst.tile([P, 4, SUB], F32)
            for j in range(4):
                nc.vector.tensor_copy(iota4[:, j, :], iota_i[:])
            epsc = cst.tile([P, 1], F32)
            nc.vector.memset(epsc[:], BN_EPS * 0.36)

            def load_res(nm, dt=None):
                t = res.tile(list(t_in[nm].shape), dt or t_in[nm].dtype,
                             tag="res_" + nm)
                nc.sync.dma_start(t[:], t_in[nm][:])
                return t

            def act_silu(out, in_, scale=1.0, bias=0.0, accum_out=None,
                         pool=None):
                if not SIM_MODE:
                    nc.scalar.activation(out[:] if hasattr(out, 'tile') else out,
                                         in_, AF.Silu, scale=scale, bias=bias,
                                         accum_out=accum_out)
                    return
                # CoreSim fallback: silu(z) = z * sigmoid(z)
                pool = pool or wrk
                shp = list(out.shape)
                sg = pool.tile(shp, F32, tag="simsg")
                nc.scalar.activation(sg[:], in_, AF.Sigmoid, scale=scale,
                                     bias=bias)
                pre = pool.tile(shp, F32, tag="simpre")
                if isinstance(scale, float) and isinstance(bias, float):
                    nc.vector.tensor_scalar(pre[:], in_, scale, bias, OP.mult,
                                            OP.add)
                else:
                    nc.vector.tensor_scalar(pre[:], in_, scale, bias,
                                            OP.mult, OP.add)
                nc.vector.tensor_mul(out, pre[:], sg[:])
                if accum_out is not None:
                    nc.vector.reduce_sum(accum_out, out,
                                         axis=mybir.AxisListType.X)

            a_idx = load_res("a_idx"); b_idx = load_res("b_idx")
            col4 = load_res("col4"); bases_sb = load_res("bases")
            w1chiT = load_res("w1chiT"); w1cloT = load_res("w1cloT")
            ew2T_sb = res.tile([P, P], ew2_dt)
            if EW2_F32R:
                tmp_w = wrk.tile([P, P], F32)
                nc.sync.dma_start(tmp_w[:], t_in["ew2T"][:])
                nc.vector.tensor_copy(ew2T_sb[:], tmp_w[:])
            else:
                nc.sync.dma_start(ew2T_sb[:], t_in["ew2T"][:])
            awcol = load_res("awcol")
            pvec = load_res("pvec"); corr = load_res("corr")

            agg = res.tile([P, NODES_PAD], F32)
            nc.vector.memset(agg[:], 0.0)
            sum_acc = res.tile([P, 1], F32)
            nc.vector.memset(sum_acc[:], 0.0)
            sq_acc = res.tile([P, 1], F32)
            nc.vector.memset(sq_acc[:], 0.0)

            # ---- phase 1: edge pass, BN stats, x to DRAM ----
            with tc.tile_pool(name="ps1", bufs=2, space="PSUM") as ps1:
                for ch in range(NCHT):
                    g = ch // (NCHT // 2)
                    isl = slice(ch * 8 * 4, (ch + 1) * 8 * 4)  # 32 cols/chunk
                    gts = []
                    for tab, idx in ((t_in["Ahi"], a_idx), (t_in["Alo"], a_idx),
                                     (t_in["Bhi"], b_idx), (t_in["Blo"], b_idx)):
                        ap = tab[:]
                        if tab.shape[0] == 2 * HALF_T:
                            ap = tab[g * HALF_T:(g + 1) * HALF_T]
                        gt = wrk.tile([P, 1, CH], BF16, tag="gt%d" % len(gts))
                        nc.gpsimd.dma_gather(gt[:], ap, idx[:, isl], CH, CH, P,
                                             transpose=True)
                        gts.append(gt)
                    eahi = wrk.tile([16, CH], BF16, tag="eahi")
                    ealo = wrk.tile([16, CH], BF16, tag="ealo")
                    nc.sync.dma_start(eahi[:], t_in["eahiT"][:, ch * CH:(ch + 1) * CH])
                    nc.sync.dma_start(ealo[:], t_in["ealoT"][:, ch * CH:(ch + 1) * CH])

                    pre1 = ps1.tile([P, CH], F32, space="PSUM", tag="pre1")
                    for i, gt in enumerate(gts):
                        nc.tensor.matmul(pre1[:], lhsT=ident_b[:],
                                         rhs=gt[:, 0, :],
                                         start=(i == 0), stop=False)
                    nc.tensor.matmul(pre1[:], lhsT=w1chiT[:], rhs=eahi[:],
                                     start=False, stop=False)
                    nc.tensor.matmul(pre1[:], lhsT=w1chiT[:], rhs=ealo[:],
                                     start=False, stop=False)
                    nc.tensor.matmul(pre1[:], lhsT=w1cloT[:], rhs=eahi[:],
                                     start=False, stop=True)

                    x_sb = wrk.tile([P, CH], F32, tag="x")
                    s_t = wrk.tile([P, 1], F32, tag="st")
                    act_silu(x_sb[:], pre1[:], accum_out=s_t[:])
                    sqd = ps1.tile([P, CH], F32, space="PSUM", tag="sqd")
                    q_t = wrk.tile([P, 1], F32, tag="qt")
                    nc.scalar.activation(sqd[:], x_sb[:], AF.Square,
                                         accum_out=q_t[:])
                    nc.vector.tensor_add(sum_acc[:], sum_acc[:], s_t[:])
                    nc.vector.tensor_add(sq_acc[:], sq_acc[:], q_t[:])
                    nc.sync.dma_start(xbuf[:, ch * CH:(ch + 1) * CH], x_sb[:])

            # ---- BN stats allreduce + affine fold ----
            nc.vector.tensor_sub(sum_acc[:], sum_acc[:], corr[:, 0:1])
            nc.vector.tensor_sub(sq_acc[:], sq_acc[:], corr[:, 1:2])
            st_pack = wrk.tile([P, 2], F32, tag="stp")
            nc.vector.tensor_copy(st_pack[:, 0:1], sum_acc[:])
            nc.vector.tensor_copy(st_pack[:, 1:2], sq_acc[:])
            cc_in = dr.tile([P, 2], F32)
            cc_out = dr.tile([P, 2], F32)
            nc.sync.dma_start(cc_in[:], st_pack[:])
            nc.gpsimd.collective_compute(
                "AllReduce", OP.add, replica_groups=[list(range(NC_CORES))],
                ins=[cc_in.opt()], outs=[cc_out.opt()])
            st_g = wrk.tile([P, 2], F32, tag="stg")
            nc.sync.dma_start(st_g[:], cc_out[:])
            mean = res.tile([P, 1], F32)
            var = res.tile([P, 1], F32)
            nc.vector.tensor_scalar_mul(mean[:], st_g[:, 0:1], 1.0 / E_total)
            nc.vector.tensor_scalar_mul(var[:], st_g[:, 1:2], 1.0 / E_total)
            m2 = wrk.tile([P, 1], F32, tag="m2")
            nc.vector.tensor_mul(m2[:], mean[:], mean[:])
            nc.vector.tensor_sub(var[:], var[:], m2[:])
            sd = wrk.tile([P, 1], F32, tag="sd")
            nc.scalar.activation(sd[:], var[:], AF.Sqrt, bias=epsc[:])
            inv = wrk.tile([P, 1], F32, tag="inv")
            nc.vector.reciprocal(inv[:], sd[:])
            gam2 = res.tile([P, 1], F32)
            beta2 = res.tile([P, 1], F32)
            nc.vector.tensor_mul(gam2[:], inv[:], pvec[:, 0:1])
            tmpb = wrk.tile([P, 1], F32, tag="tmpb")
            nc.vector.tensor_mul(tmpb[:], mean[:], gam2[:])
            nc.vector.tensor_sub(beta2[:], pvec[:, 1:2], tmpb[:])

            # ---- phase 2: edge pass 2 + scatter ----
            with (
                tc.tile_pool(name="psA", bufs=2, space="PSUM") as psA,
                tc.tile_pool(name="psB", bufs=2, space="PSUM") as psB,
                tc.tile_pool(name="psC", bufs=2, space="PSUM") as psC,
                tc.tile_pool(name="psD", bufs=1, space="PSUM") as psD,
            ):
                for ch in range(NCHT):
                    x_sb = wrk.tile([P, CH], F32, tag="x2")
                    nc.sync.dma_start(x_sb[:], xbuf[:, ch * CH:(ch + 1) * CH])
                    y_sb = wrk.tile([P, CH], ew2_dt, tag="y")
                    nc.scalar.activation(y_sb[:], x_sb[:], AF.Identity,
                                         scale=gam2[:], bias=beta2[:])
                    pre2 = psA.tile([P, CH], F32, space="PSUM", tag="pre2")
                    nc.tensor.matmul(pre2[:], lhsT=ew2T_sb[:], rhs=y_sb[:],
                                     start=True, stop=True)
                    ef = wrk.tile([P, CH], F32, tag="ef")
                    act_silu(ef[:], pre2[:], bias=pvec[:, 2:3])
                    # attention: per-subchunk matmul vs aw, sigmoid
                    attp = psC.tile([P, 4], F32, space="PSUM", tag="attp")
                    for j in range(4):
                        nc.tensor.matmul(attp[:, j:j + 1],
                                         lhsT=ef[:, j * SUB:(j + 1) * SUB],
                                         rhs=awcol[:], start=True, stop=True)
                    att = wrk.tile([P, 4], F32, tag="att")
                    nc.scalar.activation(att[:], attp[:], AF.Sigmoid,
                                         bias=pvec[:, 5:6])
                    # transpose subchunks, gate, one-hot scatter
                    tps = psB.tile([P, CH], F32, space="PSUM", tag="tps")
                    for j in range(4):
                        nc.tensor.transpose(tps[:, j * SUB:(j + 1) * SUB],
                                            ef[:, j * SUB:(j + 1) * SUB],
                                            ident_f[:])
                    gated = wrk.tile([P, CH], F32, tag="gated")
                    for j in range(4):
                        nc.vector.tensor_scalar(
                            gated[:, j * SUB:(j + 1) * SUB],
                            tps[:, j * SUB:(j + 1) * SUB],
                            att[:, j:j + 1], None, OP.mult)
                    pc = wrk.tile([P, 4, SUB], F32, tag="pc")
                    nc.vector.tensor_tensor(
                        out=pc[:],
                        in0=col4[:, ch * 4:(ch + 1) * 4][:, :, None]
                            .to_broadcast([P, 4, SUB]),
                        in1=iota4[:], op=OP.is_equal)
                    aggp = psD.tile([P, SUB], F32, space="PSUM", tag="aggp")
                    for j in range(4):
                        nc.tensor.matmul(aggp[:],
                                         lhsT=gated[:, j * SUB:(j + 1) * SUB],
                                         rhs=pc[:, j, :],
                                         start=(j == 0), stop=(j == 3))
                    base_v = nc.values_load(bases_sb[0:1, ch:ch + 1],
                                            engines=[mybir.EngineType.DVE],
                                            min_val=0,
                                            max_val=NODES_PAD - SUB,
                                            skip_runtime_bounds_check=True)
                    sl = bass.ds(base_v, SUB)
                    nc.vector.tensor_tensor(out=agg[:, sl], in0=agg[:, sl],
                                            in1=aggp[:], op=OP.add)

            # ---- phase 3: node MLP + residual ----
            nw1aT = load_res("nw1aT"); nw1bT = load_res("nw1bT")
            nw2T = load_res("nw2T")
            with tc.tile_pool(name="ps3", bufs=2, space="PSUM") as ps3:
                nt = NODES_PAD // CH
                widths = [CH] * nt + ([NODES_PAD - nt * CH]
                                      if NODES_PAD % CH else [])
                off = 0
                for w in widths:
                    hT_t = wrk.tile([P, w], F32, tag="hT")
                    nc.sync.dma_start(hT_t[:], t_in["hT"][:, off:off + w])
                    z1p = ps3.tile([P, w], F32, space="PSUM", tag="z1p")
                    nc.tensor.matmul(z1p[:], lhsT=nw1aT[:], rhs=hT_t[:],
                                     start=True, stop=False)
                    nc.tensor.matmul(z1p[:], lhsT=nw1bT[:],
                                     rhs=agg[:, off:off + w],
                                     start=False, stop=True)
                    z1 = wrk.tile([P, w], F32, tag="z1")
                    act_silu(z1[:], z1p[:], bias=pvec[:, 3:4])
                    op2 = ps3.tile([P, w], F32, space="PSUM", tag="op2")
                    nc.tensor.matmul(op2[:], lhsT=nw2T[:], rhs=z1[:],
                                     start=True, stop=True)
                    o1 = wrk.tile([P, w], F32, tag="o1")
                    nc.scalar.activation(o1[:], op2[:], AF.Identity,
                                         bias=pvec[:, 4:5])
                    o2 = wrk.tile([P, w], F32, tag="o2")
                    nc.vector.tensor_add(o2[:], o1[:], hT_t[:])
                    nc.sync.dma_start(outT[:, off:off + w], o2[:])
                    off += w
    nc.compile()
    return nc


def prepare(h, edge_index, edge_attr, ew1, eb1, gamma, beta, ew2, eb2,
            aw, ab, nw1, nb1, nw2, nb2):
    h = np.asarray(h, np.float32)
    edge_index = np.asarray(edge_index)
    edge_attr = np.asarray(edge_attr, np.float32)
    (ew1, eb1, gamma, beta, ew2, eb2, aw, ab, nw1, nb1, nw2, nb2) = [
        np.asarray(a, np.float32)
        for a in (ew1, eb1, gamma, beta, ew2, eb2, aw, ab, nw1, nb1, nw2, nb2)]

    N, D = h.shape
    E = edge_index.shape[1]
    DE = edge_attr.shape[1]
    NPC = N // NC_CORES
    HALF = N // 2
    assert D == 128 and N % NC_CORES == 0 and N // 2 <= 32767
    row = edge_index[0].astype(np.int64)
    col = edge_index[1].astype(np.int64)

    # host precompute (node-level linear projections of first edge-MLP layer)
    W1a, W1b, W1c = ew1[:, :D], ew1[:, D:2 * D], ew1[:, 2 * D:]
    A_full = (h @ W1a.T + eb1).astype(np.float32)
    B_full = (h @ W1b.T).astype(np.float32)

    # shard + sort
    core = col // NPC
    grp = (row >= HALF).astype(np.int64)
    orders = []
    G = 0
    for c in range(NC_CORES):
        po = []
        for g in (0, 1):
            m = np.nonzero((core == c) & (grp == g))[0]
            po.append(m[np.argsort(col[m], kind="stable")])
            G = max(G, po[-1].shape[0])
        orders.append(po)
    G = (G + CH - 1) // CH * CH
    NODES_PAD = (NPC + P - 1) // P * P

    w1chiT_, w1cloT_ = _hilo(np.ascontiguousarray(W1c.T))
    Bhi, Blo = _hilo(B_full)
    pvec_c = np.zeros((P, 6), np.float32)
    pvec_c[:, 0] = gamma; pvec_c[:, 1] = beta; pvec_c[:, 2] = eb2
    pvec_c[:, 3] = nb1; pvec_c[:, 4] = nb2; pvec_c[:, 5] = ab[0]
    shared = dict(
        Bhi=np.ascontiguousarray(Bhi), Blo=np.ascontiguousarray(Blo),
        w1chiT=np.ascontiguousarray(w1chiT_),
        w1cloT=np.ascontiguousarray(w1cloT_),
        ew2T=np.ascontiguousarray(ew2.T),
        awcol=np.ascontiguousarray((aw[0] * SS)[:, None]),
        nw1aT=np.ascontiguousarray(nw1[:, :D].T),
        nw1bT=np.ascontiguousarray((nw1[:, D:] * (SS / NORM)).T),
        nw2T=np.ascontiguousarray((nw2 * SS).T),
        pvec=pvec_c,
    )
    in_maps = []
    for c in range(NC_CORES):
        d = prep_core(c, col, row, orders[c], h, edge_attr, A_full, B_full,
                      G, NPC, HALF, NODES_PAD)
        hT = np.zeros((P, NODES_PAD), np.float32)
        hT[:, :NPC] = h[c * NPC:(c + 1) * NPC].T
        d["hT"] = hT
        d.update(shared)
        in_maps.append(d)

    nc = build_nc(G, NPC, NODES_PAD, float(E), HALF)
    return nc, in_maps, (N, D, NPC)


def finalize(results, meta):
    N, D, NPC = meta
    out = np.empty((N, D), np.float32)
    for c in range(NC_CORES):
        out[c * NPC:(c + 1) * NPC] = results[c]["outT"][:, :NPC].T
    return out


def kernel(**inputs):
    nc, in_maps, meta = prepare(**inputs)
    if SIM_MODE:
        import concourse.bass_interp as bass_interp
        sim = bass_interp.MultiCoreSim(nc, NC_CORES)
        for c in range(NC_CORES):
            for nm, v in in_maps[c].items():
                sim.cores[c].tensor(nm)[:] = v
        sim.simulate()
        return finalize([{ "outT": sim.cores[c].tensor("outT") }
                         for c in range(NC_CORES)], meta)
    res = run_bass_kernel_spmd(nc, in_maps, list(range(NC_CORES)))
    return finalize([res.results[c] for c in range(NC_CORES)], meta)


# revision 12
# speedup vs baseline: 1.1723x; 1.1723x over previous
"""Trainium2 Bass kernel for nn_GCL (GNN message-passing layer), 8-core SPMD.

Sharding: edges sharded by destination node (col) range; each core owns
N/8 = 6250 nodes and all edges pointing at them. Within a core, edges are
split into 2 groups by source-row half (so gather indices fit int16), each
group sorted by col. BatchNorm batch stats are all-reduced across cores.

Device pipeline per core:
  P1  per 512-edge chunk: 4 transposed bf16 dma_gathers (A[col], B[row] as
      exact hi/lo bf16 pairs) -> identity-matmul PSUM accumulation
      + edge_attr @ W1c (hi/lo bf16 matmuls) -> Silu (ScalarE, accum_out
      gives BN sum) -> Square (accum_out gives BN sqsum) -> x to DRAM.
  AR  tiny AllReduce of [128,2] BN stats; fold into per-feature scale/bias.
  P2  reload x -> fused BN-affine+Silu -> ew2 matmul -> Silu -> attention
      (per-subchunk matmul + Sigmoid) -> PE transpose -> gate -> one-hot
      matmul scatter into PSUM -> dynamic-offset add into SBUF agg.
  P3  node MLP feature-major + residual; output transposed per core.

A = h @ W1a.T + eb1 and B = h @ W1b.T are per-node precomputes (host, exact
fp32, shipped as hi/lo bf16 pairs: error ~2^-17). All 1/0.6 activation
scales are folded into BN/eps/weights exactly (BN is scale-invariant).
"""
import os
import numpy as np
import ml_dtypes

import concourse.bass as bass
import concourse.bacc as bacc
import concourse.mybir as mybir
import concourse.tile as tile
from concourse.bass_utils import run_bass_kernel_spmd
from concourse.masks import make_identity

F32 = mybir.dt.float32
F32R = mybir.dt.float32r
BF16 = mybir.dt.bfloat16
I16 = mybir.dt.int16
I32 = mybir.dt.int32
AF = mybir.ActivationFunctionType
OP = mybir.AluOpType

NC_CORES = 8
P = 128
CH = 512          # edges per chunk
SUB = 128         # edges per subchunk
BN_EPS = 1e-5
NORM = 100.0
SS = 1.0 / 0.6    # scaled-silu factor

EW2_F32R = os.environ.get("EW2_F32R", "0") == "1"
SIM_MODE = os.environ.get("BASS_KERNEL_SIM", "0") == "1"
SKIP_AR = os.environ.get("SKIP_AR", "0") == "1"


def _hilo(x):
    hi = x.astype(ml_dtypes.bfloat16)
    lo = (x - hi.astype(np.float32)).astype(ml_dtypes.bfloat16)
    return hi, lo


def _silu(x):
    return x / (1.0 + np.exp(-x))


def _pack_idx(idx):
    """int16 [n] -> [128, n//16] wrapped in 16 partitions, replicated x8."""
    n = idx.shape[0]
    w = idx.reshape(n // 16, 16).T            # [16, n//16]
    return np.tile(w, (8, 1)).astype(np.int16)


def prep_core(c, col, row, order_all, h, edge_attr, A_full, B_full, G, NPC, HALF, NODES_PAD):
    """Build per-core host tensors. order_all: edge ids for core c,
    list of 2 arrays (per group), each sorted by col."""
    S = 2 * G
    nch_g = G // CH
    colloc = np.full(S, 0, np.int64)
    rowloc = np.zeros(S, np.int64)
    col_rel = np.full(S, -1000.0, np.float64)
    bases = np.zeros(2 * nch_g, np.int64)
    n_pad = np.zeros(2, np.int64)
    ea_s = np.zeros((S, edge_attr.shape[1]), np.float32)
    for g in (0, 1):
        ids = order_all[g]
        n = ids.shape[0]
        o = g * G
        n_pad[g] = G - n
        colloc[o:o + n] = col[ids] - c * NPC
        rowloc[o:o + n] = row[ids] - g * HALF
        ea_s[o:o + n] = edge_attr[ids]
        for k in range(nch_g):
            s = o + k * CH
            ncheck = min(n - k * CH, CH)
            if ncheck <= 0:
                base = 0
            else:
                base = min(colloc[s], NODES_PAD - SUB)
                span = colloc[s + ncheck - 1] - base
                assert span < P, f"chunk col_rel {span} >= {P}"
            bases[g * nch_g + k] = base
            col_rel[s:s + CH] = colloc[s:s + CH] - base
        col_rel[o + n:o + G] = -1000.0
    # pads gather table row 0
    a_idx = colloc.copy()
    b_idx = rowloc.copy()
    for g in (0, 1):
        o = g * G
        n = G - int(n_pad[g])
        a_idx[o + n:o + G] = 0
        b_idx[o + n:o + G] = 0

    # exact stats corrections for pad edges (mimic device op order)
    corr_sum = np.zeros(P, np.float32)
    corr_sq = np.zeros(P, np.float32)
    Ahi, Alo = _hilo(A_full[c * NPC:(c + 1) * NPC])
    Bhi, Blo = _hilo(B_full)
    for g in (0, 1):
        if n_pad[g] == 0:
            continue
        a0 = Ahi[0].astype(np.float32) + Alo[0].astype(np.float32)
        b0 = (Bhi[g * HALF].astype(np.float32) + Blo[g * HALF].astype(np.float32))
        xp = _silu((a0 + b0).astype(np.float32)).astype(np.float32)
        corr_sum += n_pad[g] * xp
        corr_sq += n_pad[g] * xp * xp

    eahi, ealo = _hilo(ea_s)
    return dict(
        a_idx=_pack_idx(a_idx.astype(np.int16)),
        b_idx=_pack_idx(b_idx.astype(np.int16)),
        col4=col_rel.reshape(S // SUB, SUB).T.astype(np.float32).copy(),  # [128, nsub]
        bases=bases[None, :].astype(np.int32),
        eahiT=np.ascontiguousarray(eahi.T),      # [16, S] bf16
        ealoT=np.ascontiguousarray(ealo.T),
        Ahl=np.ascontiguousarray(np.concatenate([Ahi, Alo], 1)),
        corr=np.stack([corr_sum, corr_sq], 1),   # [128, 2]
    )


def build_nc(G, NPC, NODES_PAD, E_total, HALF_T):
    S = 2 * G
    NCHT = S // CH
    NSUB = S // SUB
    nc = bacc.Bacc("TRN2", target_bir_lowering=False, debug=False,
                   num_devices=NC_CORES)
    t_in = {}
    for nm, shp, dt in [
        ("Ahl", [NPC, 2 * P], BF16),
        ("Bhl", [2 * HALF_T, 2 * P], BF16),
        ("eahiT", [16, S], BF16), ("ealoT", [16, S], BF16),
        ("a_idx", [P, NSUB * 8], I16), ("b_idx", [P, NSUB * 8], I16),
        ("col4", [P, NSUB], F32), ("bases", [1, NCHT], I32),
        ("corr", [P, 2], F32),
        ("w1chiT", [16, P], BF16), ("w1cloT", [16, P], BF16),
        ("ew2T", [P, P], F32), ("awcol", [P, 1], F32),
        ("nw1aT", [P, P], F32), ("nw1bT", [P, P], F32), ("nw2T", [P, P], F32),
        ("pvec", [P, 6], F32),   # 0 gamma,1 beta,2 eb2,3 nb1,4 nb2,5 ab_rep
        ("hT", [P, NODES_PAD], F32),
    ]:
        t_in[nm] = nc.dram_tensor(nm, shp, dt, kind="ExternalInput")
    outT = nc.dram_tensor("outT", [P, NODES_PAD], F32, kind="ExternalOutput")
    xbuf = nc.dram_tensor("xbuf", [P, S], F32)

    ew2_dt = F32R if EW2_F32R else F32

    with tile.TileContext(nc) as tc:
        with (
            tc.tile_pool(name="cst", bufs=1) as cst,
            tc.tile_pool(name="res", bufs=1) as res,
            tc.tile_pool(name="wrk", bufs=3) as wrk,
            tc.tile_pool(name="dram", bufs=1, space="DRAM") as dr,
        ):
            # ---- constants / resident tensors ----
            ident_b = cst.tile([P, P], BF16)
            make_identity(nc, ident_b[:])
            ident_f = cst.tile([P, P], F32)
            make_identity(nc, ident_f[:])
            iota_i = cst.tile([P, SUB], I32)
            nc.gpsimd.iota(iota_i[:], pattern=[[1, SUB]], base=0,
                           channel_multiplier=0)
            iota4 = cst.tile([P, 4, SUB], F32)
            for j in range(4):
                nc.vector.tensor_copy(iota4[:, j, :], iota_i[:])
            epsc = cst.tile([P, 1], F32)
            nc.vector.memset(epsc[:], BN_EPS * 0.36)

            def load_res(nm, dt=None):
                t = res.tile(list(t_in[nm].shape), dt or t_in[nm].dtype,
                             tag="res_" + nm)
                nc.sync.dma_start(t[:], t_in[nm][:])
                return t

            def act_silu(out, in_, scale=1.0, bias=0.0, accum_out=None,
                         pool=None):
                if not SIM_MODE:
                    nc.scalar.activation(out[:] if hasattr(out, 'tile') else out,
                                         in_, AF.Silu, scale=scale, bias=bias,
                                         accum_out=accum_out)
                    return
                # CoreSim fallback: silu(z) = z * sigmoid(z)
                pool = pool or wrk
                shp = list(out.shape)
                sg = pool.tile(shp, F32, tag="simsg")
                nc.scalar.activation(sg[:], in_, AF.Sigmoid, scale=scale,
                                     bias=bias)
                pre = pool.tile(shp, F32, tag="simpre")
                if isinstance(scale, float) and isinstance(bias, float):
                    nc.vector.tensor_scalar(pre[:], in_, scale, bias, OP.mult,
                                            OP.add)
                else:
                    nc.vector.tensor_scalar(pre[:], in_, scale, bias,
                                            OP.mult, OP.add)
                nc.vector.tensor_mul(out, pre[:], sg[:])
                if accum_out is not None:
                    nc.vector.reduce_sum(accum_out, out,
                                         axis=mybir.AxisListType.X)

            a_idx = load_res("a_idx"); b_idx = load_res("b_idx")
            col4 = load_res("col4"); bases_sb = load_res("bases")
            w1chiT = load_res("w1chiT"); w1cloT = load_res("w1cloT")
            ew2T_sb = res.tile([P, P], ew2_dt)
            if EW2_F32R:
                tmp_w = wrk.tile([P, P], F32)
                nc.sync.dma_start(tmp_w[:], t_in["ew2T"][:])
                nc.vector.tensor_copy(ew2T_sb[:], tmp_w[:])
            else:
                nc.sync.dma_start(ew2T_sb[:], t_in["ew2T"][:])
            awcol = load_res("awcol")
            pvec = load_res("pvec"); corr = load_res("corr")

            agg = res.tile([P, NODES_PAD], F32)
            nc.vector.memset(agg[:], 0.0)
            sum_acc = res.tile([P, 1], F32)
            nc.vector.memset(sum_acc[:], 0.0)
            sq_acc = res.tile([P, 1], F32)
            nc.vector.memset(sq_acc[:], 0.0)

            # ---- phase 1: edge pass, BN stats, x to DRAM ----
            def _p1_chunk(ch, gts):
                    eahi = wrk.tile([16, CH], BF16, tag="eahi")
                    ealo = wrk.tile([16, CH], BF16, tag="ealo")
                    nc.sync.dma_start(eahi[:], t_in["eahiT"][:, ch * CH:(ch + 1) * CH])
                    nc.sync.dma_start(ealo[:], t_in["ealoT"][:, ch * CH:(ch + 1) * CH])

                    pre1 = ps1.tile([P, CH], F32, space="PSUM", tag="pre1")
                    for i, gt in enumerate(gts):
                        nc.tensor.matmul(pre1[:], lhsT=ident_b[:],
                                         rhs=gt,
                                         start=(i == 0), stop=False)
                    nc.tensor.matmul(pre1[:], lhsT=w1chiT[:], rhs=eahi[:],
                                     start=False, stop=False)
                    nc.tensor.matmul(pre1[:], lhsT=w1chiT[:], rhs=ealo[:],
                                     start=False, stop=False)
                    nc.tensor.matmul(pre1[:], lhsT=w1cloT[:], rhs=eahi[:],
                                     start=False, stop=True)

                    x_sb = wrk.tile([P, CH], F32, tag="x")
                    s_t = wrk.tile([P, 1], F32, tag="st")
                    act_silu(x_sb[:], pre1[:], accum_out=s_t[:])
                    sqd = ps1.tile([P, CH], F32, space="PSUM", tag="sqd")
                    q_t = wrk.tile([P, 1], F32, tag="qt")
                    nc.scalar.activation(sqd[:], x_sb[:], AF.Square,
                                         accum_out=q_t[:])
                    nc.vector.tensor_add(sum_acc[:], sum_acc[:], s_t[:])
                    nc.vector.tensor_add(sq_acc[:], sq_acc[:], q_t[:])
                    nc.sync.dma_start(xbuf[:, ch * CH:(ch + 1) * CH], x_sb[:])

            GB = int(os.environ.get("GATHER_BATCH", "4"))
            nch_g = NCHT // 2
            with tc.tile_pool(name="ps1", bufs=2, space="PSUM") as ps1:
              for g in (0, 1):
                for kb in range(0, nch_g, GB):
                    nb = min(GB, nch_g - kb)
                    NI = nb * CH
                    chb = g * nch_g + kb
                    isl = slice(chb * 32, (chb + nb) * 32)
                    gA = wrk.tile([P, 2, NI], BF16, tag="gA")
                    nc.gpsimd.dma_gather(gA[:], t_in["Ahl"][:],
                                         a_idx[:, isl], NI, NI, 2 * P,
                                         transpose=True)
                    gB = wrk.tile([P, 2, NI], BF16, tag="gB")
                    nc.gpsimd.dma_gather(
                        gB[:], t_in["Bhl"][g * HALF_T:(g + 1) * HALF_T],
                        b_idx[:, isl], NI, NI, 2 * P, transpose=True)
                    for k in range(nb):
                        ch = chb + k
                        _p1_chunk(ch,
                                  [gA[:, 0, k * CH:(k + 1) * CH],
                                   gA[:, 1, k * CH:(k + 1) * CH],
                                   gB[:, 0, k * CH:(k + 1) * CH],
                                   gB[:, 1, k * CH:(k + 1) * CH]])

            # ---- BN stats allreduce + affine fold ----
            nc.vector.tensor_sub(sum_acc[:], sum_acc[:], corr[:, 0:1])
            nc.vector.tensor_sub(sq_acc[:], sq_acc[:], corr[:, 1:2])
            st_pack = wrk.tile([P, 2], F32, tag="stp")
            nc.vector.tensor_copy(st_pack[:, 0:1], sum_acc[:])
            nc.vector.tensor_copy(st_pack[:, 1:2], sq_acc[:])
            cc_in = dr.tile([P, 2], F32)
            cc_out = dr.tile([P, 2], F32)
            nc.sync.dma_start(cc_in[:], st_pack[:])
            if SKIP_AR:
                nc.sync.dma_start(cc_out[:], cc_in[:])
            else:
                nc.gpsimd.collective_compute(
                    "AllReduce", OP.add,
                    replica_groups=[list(range(NC_CORES))],
                    ins=[cc_in.opt()], outs=[cc_out.opt()])
            st_g = wrk.tile([P, 2], F32, tag="stg")
            nc.sync.dma_start(st_g[:], cc_out[:])
            mean = res.tile([P, 1], F32)
            var = res.tile([P, 1], F32)
            nc.vector.tensor_scalar_mul(mean[:], st_g[:, 0:1], 1.0 / E_total)
            nc.vector.tensor_scalar_mul(var[:], st_g[:, 1:2], 1.0 / E_total)
            m2 = wrk.tile([P, 1], F32, tag="m2")
            nc.vector.tensor_mul(m2[:], mean[:], mean[:])
            nc.vector.tensor_sub(var[:], var[:], m2[:])
            sd = wrk.tile([P, 1], F32, tag="sd")
            nc.scalar.activation(sd[:], var[:], AF.Sqrt, bias=epsc[:])
            inv = wrk.tile([P, 1], F32, tag="inv")
            nc.vector.reciprocal(inv[:], sd[:])
            gam2 = res.tile([P, 1], F32)
            beta2 = res.tile([P, 1], F32)
            nc.vector.tensor_mul(gam2[:], inv[:], pvec[:, 0:1])
            tmpb = wrk.tile([P, 1], F32, tag="tmpb")
            nc.vector.tensor_mul(tmpb[:], mean[:], gam2[:])
            nc.vector.tensor_sub(beta2[:], pvec[:, 1:2], tmpb[:])

            # ---- phase 2: edge pass 2 + scatter ----
            with (
                tc.tile_pool(name="psA", bufs=2, space="PSUM") as psA,
                tc.tile_pool(name="psB", bufs=2, space="PSUM") as psB,
                tc.tile_pool(name="psC", bufs=2, space="PSUM") as psC,
                tc.tile_pool(name="psD", bufs=1, space="PSUM") as psD,
            ):
                for ch in range(NCHT):
                    x_sb = wrk.tile([P, CH], F32, tag="x2")
                    nc.sync.dma_start(x_sb[:], xbuf[:, ch * CH:(ch + 1) * CH])
                    y_sb = wrk.tile([P, CH], ew2_dt, tag="y")
                    nc.scalar.activation(y_sb[:], x_sb[:], AF.Identity,
                                         scale=gam2[:], bias=beta2[:])
                    pre2 = psA.tile([P, CH], F32, space="PSUM", tag="pre2")
                    nc.tensor.matmul(pre2[:], lhsT=ew2T_sb[:], rhs=y_sb[:],
                                     start=True, stop=True)
                    ef = wrk.tile([P, CH], F32, tag="ef")
                    act_silu(ef[:], pre2[:], bias=pvec[:, 2:3])
                    # attention: per-subchunk matmul vs aw, sigmoid
                    attp = psC.tile([P, 4], F32, space="PSUM", tag="attp")
                    for j in range(4):
                        nc.tensor.matmul(attp[:, j:j + 1],
                                         lhsT=ef[:, j * SUB:(j + 1) * SUB],
                                         rhs=awcol[:], start=True, stop=True)
                    att = wrk.tile([P, 4], F32, tag="att")
                    nc.scalar.activation(att[:], attp[:], AF.Sigmoid,
                                         bias=pvec[:, 5:6])
                    # transpose subchunks, gate, one-hot scatter
                    tps = psB.tile([P, CH], F32, space="PSUM", tag="tps")
                    for j in range(4):
                        nc.tensor.transpose(tps[:, j * SUB:(j + 1) * SUB],
                                            ef[:, j * SUB:(j + 1) * SUB],
                                            ident_f[:])
                    gated = wrk.tile([P, CH], F32, tag="gated")
                    for j in range(4):
                        nc.vector.tensor_scalar(
                            gated[:, j * SUB:(j + 1) * SUB],
                            tps[:, j * SUB:(j + 1) * SUB],
                            att[:, j:j + 1], None, OP.mult)
                    pc = wrk.tile([P, 4, SUB], F32, tag="pc")
                    nc.vector.tensor_tensor(
                        out=pc[:],
                        in0=col4[:, ch * 4:(ch + 1) * 4][:, :, None]
                            .to_broadcast([P, 4, SUB]),
                        in1=iota4[:], op=OP.is_equal)
                    aggp = psD.tile([P, SUB], F32, space="PSUM", tag="aggp")
                    for j in range(4):
                        nc.tensor.matmul(aggp[:],
                                         lhsT=gated[:, j * SUB:(j + 1) * SUB],
                                         rhs=pc[:, j, :],
                                         start=(j == 0), stop=(j == 3))
                    base_v = nc.values_load(bases_sb[0:1, ch:ch + 1],
                                            engines=[mybir.EngineType.DVE],
                                            min_val=0,
                                            max_val=NODES_PAD - SUB,
                                            skip_runtime_bounds_check=True)
                    sl = bass.ds(base_v, SUB)
                    nc.vector.tensor_tensor(out=agg[:, sl], in0=agg[:, sl],
                                            in1=aggp[:], op=OP.add)

            # ---- phase 3: node MLP + residual ----
            nw1aT = load_res("nw1aT"); nw1bT = load_res("nw1bT")
            nw2T = load_res("nw2T")
            with tc.tile_pool(name="ps3", bufs=2, space="PSUM") as ps3:
                nt = NODES_PAD // CH
                widths = [CH] * nt + ([NODES_PAD - nt * CH]
                                      if NODES_PAD % CH else [])
                off = 0
                for w in widths:
                    hT_t = wrk.tile([P, w], F32, tag="hT")
                    nc.sync.dma_start(hT_t[:], t_in["hT"][:, off:off + w])
                    z1p = ps3.tile([P, w], F32, space="PSUM", tag="z1p")
                    nc.tensor.matmul(z1p[:], lhsT=nw1aT[:], rhs=hT_t[:],
                                     start=True, stop=False)
                    nc.tensor.matmul(z1p[:], lhsT=nw1bT[:],
                                     rhs=agg[:, off:off + w],
                                     start=False, stop=True)
                    z1 = wrk.tile([P, w], F32, tag="z1")
                    act_silu(z1[:], z1p[:], bias=pvec[:, 3:4])
                    op2 = ps3.tile([P, w], F32, space="PSUM", tag="op2")
                    nc.tensor.matmul(op2[:], lhsT=nw2T[:], rhs=z1[:],
                                     start=True, stop=True)
                    o1 = wrk.tile([P, w], F32, tag="o1")
                    nc.scalar.activation(o1[:], op2[:], AF.Identity,
                                         bias=pvec[:, 4:5])
                    o2 = wrk.tile([P, w], F32, tag="o2")
                    nc.vector.tensor_add(o2[:], o1[:], hT_t[:])
                    nc.sync.dma_start(outT[:, off:off + w], o2[:])
                    off += w
    nc.compile()
    return nc


def prepare(h, edge_index, edge_attr, ew1, eb1, gamma, beta, ew2, eb2,
            aw, ab, nw1, nb1, nw2, nb2):
    h = np.asarray(h, np.float32)
    edge_index = np.asarray(edge_index)
    edge_attr = np.asarray(edge_attr, np.float32)
    (ew1, eb1, gamma, beta, ew2, eb2, aw, ab, nw1, nb1, nw2, nb2) = [
        np.asarray(a, np.float32)
        for a in (ew1, eb1, gamma, beta, ew2, eb2, aw, ab, nw1, nb1, nw2, nb2)]

    N, D = h.shape
    E = edge_index.shape[1]
    DE = edge_attr.shape[1]
    NPC = N // NC_CORES
    HALF = N // 2
    assert D == 128 and N % NC_CORES == 0 and N // 2 <= 32767
    row = edge_index[0].astype(np.int64)
    col = edge_index[1].astype(np.int64)

    # host precompute (node-level linear projections of first edge-MLP layer)
    W1a, W1b, W1c = ew1[:, :D], ew1[:, D:2 * D], ew1[:, 2 * D:]
    A_full = (h @ W1a.T + eb1).astype(np.float32)
    B_full = (h @ W1b.T).astype(np.float32)

    # shard + sort
    core = col // NPC
    grp = (row >= HALF).astype(np.int64)
    orders = []
    G = 0
    for c in range(NC_CORES):
        po = []
        for g in (0, 1):
            m = np.nonzero((core == c) & (grp == g))[0]
            po.append(m[np.argsort(col[m], kind="stable")])
            G = max(G, po[-1].shape[0])
        orders.append(po)
    G = (G + CH - 1) // CH * CH
    NODES_PAD = (NPC + P - 1) // P * P

    w1chiT_, w1cloT_ = _hilo(np.ascontiguousarray(W1c.T))
    Bhi, Blo = _hilo(B_full)
    Bhl = np.ascontiguousarray(np.concatenate([Bhi, Blo], 1))
    pvec_c = np.zeros((P, 6), np.float32)
    pvec_c[:, 0] = gamma; pvec_c[:, 1] = beta; pvec_c[:, 2] = eb2
    pvec_c[:, 3] = nb1; pvec_c[:, 4] = nb2; pvec_c[:, 5] = ab[0]
    shared = dict(
        Bhl=Bhl,
        w1chiT=np.ascontiguousarray(w1chiT_),
        w1cloT=np.ascontiguousarray(w1cloT_),
        ew2T=np.ascontiguousarray(ew2.T),
        awcol=np.ascontiguousarray((aw[0] * SS)[:, None]),
        nw1aT=np.ascontiguousarray(nw1[:, :D].T),
        nw1bT=np.ascontiguousarray((nw1[:, D:] * (SS / NORM)).T),
        nw2T=np.ascontiguousarray((nw2 * SS).T),
        pvec=pvec_c,
    )
    in_maps = []
    for c in range(NC_CORES):
        d = prep_core(c, col, row, orders[c], h, edge_attr, A_full, B_full,
                      G, NPC, HALF, NODES_PAD)
        hT = np.zeros((P, NODES_PAD), np.float32)
        hT[:, :NPC] = h[c * NPC:(c + 1) * NPC].T
        d["hT"] = hT
        d.update(shared)
        in_maps.append(d)

    nc = build_nc(G, NPC, NODES_PAD, float(E), HALF)
    return nc, in_maps, (N, D, NPC)


def finalize(results, meta):
    N, D, NPC = meta
    out = np.empty((N, D), np.float32)
    for c in range(NC_CORES):
        out[c * NPC:(c + 1) * NPC] = results[c]["outT"][:, :NPC].T
    return out


def kernel(**inputs):
    nc, in_maps, meta = prepare(**inputs)
    if SIM_MODE:
        import concourse.bass_interp as bass_interp
        sim = bass_interp.MultiCoreSim(nc, NC_CORES)
        for c in range(NC_CORES):
            for nm, v in in_maps[c].items():
                sim.cores[c].tensor(nm)[:] = v
        sim.simulate()
        return finalize([{ "outT": sim.cores[c].tensor("outT") }
                         for c in range(NC_CORES)], meta)
    res = run_bass_kernel_spmd(nc, in_maps, list(range(NC_CORES)))
    return finalize([res.results[c] for c in range(NC_CORES)], meta)
